# revision 14
# baseline (speedup 1.0000x reference)
"""Trainium2 Bass kernel for nn_FactorizedCrossAttention.

Key algebraic facts used (verified against the reference in fp64):
  * The "spatial" and "temporal" branches compute IDENTICAL per-position
    values: cross-attention over text tokens is independent per query row,
    and qt rows equal qs rows (same x row through the same Wq).  Hence
    spatial == temporal exactly.
  * concat([A, A]) @ Wst @ Wo == A @ ((Wst[:D] + Wst[D:]) @ Wo) — so both
    output projections fold into one 1024x1024 matrix Weff.
  * softmax scale (0.125) is folded into K on the host; the padding-mask
    bias is a per-token column vector applied inside the exp activation.

v2 changes vs the 345us baseline (PE-bound at 92.3%):
  * Qproj half-fp8: contraction chunks 0-3 run as 2 fp8e4 DoubleRow
    matmuls (2 k-subtiles each, 2x column rate), chunks 4-7 stay bf16;
    both accumulate into the same PSUM tile.  Empirical rel-err 0.0165
    (gate 2e-2) via exact numpy simulation of the quantization.
  * The per-rowtile reciprocal broadcast moved off the PE: rcb [16,512]
    is bounced through a DRAM scratch tensor and read back by 16
    broadcast-read DMAs (partition-replicating APs are only legal on the
    DRAM side; gpsimd partition_broadcast returns garbage on hw for most
    source/dest bases).  This removes the 8 bcast selector matmuls per
    rowtile from the PE.
  * PE warmup: dummy matmuls on a zeroed tile issue during the startup
    DMA wait so the HAM clock-gate ramps at ~11us instead of 19.5us.

PE per rowtile: Qproj 24576c + qk 8192 + sums 8192 + PV 8192 +
Wproj 32768 = 81920c (34.1us warm) vs 94208c baseline.

Sharding: pure data-parallel over (B, T_frames): 32 frames / 8 cores =
4 frames (4096 query rows) per core; K/V/weights replicated.  No
collectives.

Device layout is "transposed activations": X^T, Q^T, A^T all live as
[feature-part, row-free] tiles so every matmul is a natural slice.  Head h
occupies partitions (h%2)*64..+64 of feature chunk h//2; K^T is replicated
on both partition halves so odd heads read lane-aligned operands, and odd
heads' PV output is placed at PSUM base 64 (tile_position) so A^T lands on
partitions 64..127 without any cross-partition copies.
"""

import sys

if "/opt/trn_rl_repo" not in sys.path:
    sys.path.insert(0, "/opt/trn_rl_repo")

from contextlib import ExitStack

import ml_dtypes
import numpy as np

import concourse.bass as bass
import concourse.mybir as mybir
import concourse.tile as tile
from concourse import bacc
from concourse.bass_utils import run_bass_kernel_spmd

BF16 = ml_dtypes.bfloat16
FP8 = ml_dtypes.float8_e4m3

D = 1024           # d_model
H = 16             # num heads
G = 4              # query groups
HD = 64            # head dim
HPG = H // G       # heads per group
SCALE = 0.125
B, T, HW, TT = 2, 16, 1024, 77
NCORES = 8
FPC = (B * T) // NCORES      # frames per core = 4
ROWS = FPC * HW              # 4096 query rows per core
RT = 512                     # rows per row-tile
NRT = ROWS // RT             # 8
NK = D // 128                # 8 partition chunks of d_model
NKB = 4                      # bf16 contraction chunks (4..7)
NWARM = 12                   # PE warmup dummy matmuls

_PROG_CACHE = {}


def _patch_act_tables():
    """Force every activation onto the one table set that contains Exp, Ln
    and Copy together (natural_log_exp_and_others, same 400-interval
    precision).  Without this, bacc's table-load pass can alternate between
    table sets, costing a ~1.28us ACT_TABLE_LOAD per switch."""
    import concourse.bacc as _bm
    import concourse.hw_specs as _hw
    if getattr(_bm, "_act_tables_patched", False):
        return
    _orig = _hw.get_activation_tables

    def patched(arch):
        t = dict(_orig(arch))
        combo = None
        for name, funcs in t.items():
            if (mybir.ActivationFunctionType.Exp in funcs
                    and mybir.ActivationFunctionType.Ln in funcs):
                combo = name
                break
        if combo is not None:
            for name in list(t):
                if name != combo:
                    t[name] = set()
        return t

    _bm.get_activation_tables = patched
    _bm._act_tables_patched = True

# test.py can flip these for profiling runs
TRACE = False
TRACE_KWARGS = {}
LAST_RESULTS = None


def _build_program():
    _patch_act_tables()
    dt = mybir.dt
    DR = mybir.MatmulPerfMode.DoubleRow
    nc = bacc.Bacc("TRN2", target_bir_lowering=False, debug=False,
                   num_devices=NCORES)

    # fp8 half of X^T / Wq (contraction chunks 0-3 as 2 DoubleRow pairs)
    xt8 = nc.dram_tensor("xt8", [128, 2, 2, ROWS], dt.float8e4,
                         kind="ExternalInput").ap()
    wq8 = nc.dram_tensor("wq8", [128, NK, 2, 2, 128], dt.float8e4,
                         kind="ExternalInput").ap()
    # bf16 half (contraction chunks 4-7), chain-major
    xtb = nc.dram_tensor("xtb", [128, NKB, ROWS], dt.bfloat16,
                         kind="ExternalInput").ap()
    wqb = nc.dram_tensor("wqb", [128, NK, NKB * 128], dt.bfloat16,
                         kind="ExternalInput").ap()
    weff = nc.dram_tensor("weff", [D, D], dt.bfloat16, kind="ExternalInput").ap()
    # per-head qk blocks: [128, H*128], block h rows (h%2)*64..+64 = K_g^T
    kt = nc.dram_tensor("kt", [128, H * 128], dt.bfloat16, kind="ExternalInput").ap()
    # per-head PV blocks: [128, H*128], block h rows 0..76 = V_g at cols (h%2)*64..+64
    v64 = nc.dram_tensor("v64", [128, H * 128], dt.bfloat16, kind="ExternalInput").ap()
    # per-head one-hot sum selectors: [128, H*128], block h: col h = 1 on rows 0..76
    sel = nc.dram_tensor("sel", [128, H * 128], dt.bfloat16,
                         kind="ExternalInput").ap()
    # DRAM bounce buffer for the softmax reciprocals: row rt*16+h = head h's
    # recip for rowtile rt.  DMA reads replicate one row to 64 partitions
    # (broadcast-read APs are only legal on the DRAM side).
    rcb_d = nc.dram_tensor("rcb_d", [NRT * 16, RT], dt.bfloat16,
                           kind="Internal").ap()
    # padding-mask bias as a per-partition column vector, applied inside the
    # exp activation (func(scale*in + bias)); all-zero when the mask is all-True
    biasr = nc.dram_tensor("biasr", [128, 1], dt.float32, kind="ExternalInput").ap()
    out = nc.dram_tensor("out", [ROWS, D], dt.float32, kind="ExternalOutput").ap()

    with tile.TileContext(nc) as tc, ExitStack() as ctx:
        wpool = ctx.enter_context(tc.tile_pool(name="weights", bufs=1))
        xpool = ctx.enter_context(tc.tile_pool(name="xt", bufs=3))
        x8pool = ctx.enter_context(tc.tile_pool(name="xt8", bufs=3))
        x0pool = ctx.enter_context(tc.tile_pool(name="xt0", bufs=1))
        qpool = ctx.enter_context(tc.tile_pool(name="qt", bufs=2))
        apool = ctx.enter_context(tc.tile_pool(name="at", bufs=2))
        ppool = ctx.enter_context(tc.tile_pool(name="pt", bufs=4))
        aupool = ctx.enter_context(tc.tile_pool(name="au", bufs=9))
        rcfpool = ctx.enter_context(tc.tile_pool(name="rcf", bufs=2))
        rcbpool = ctx.enter_context(tc.tile_pool(name="rcb", bufs=2))
        rbpool = ctx.enter_context(tc.tile_pool(name="rb", bufs=3))
        opool = ctx.enter_context(tc.tile_pool(name="osb", bufs=3))
        # 8 PSUM banks: qp(2) + sp(2) + ap(2) + su(1) + op(1)
        qpsum = ctx.enter_context(tc.tile_pool(name="qpsum", bufs=2, space="PSUM"))
        spsum = ctx.enter_context(tc.tile_pool(name="spsum", bufs=2, space="PSUM"))
        apsum = ctx.enter_context(tc.tile_pool(name="apsum", bufs=2, space="PSUM"))
        supsum = ctx.enter_context(tc.tile_pool(name="supsum", bufs=1, space="PSUM"))
        opsum = ctx.enter_context(tc.tile_pool(name="opsum", bufs=1, space="PSUM"))

        # --- PE warmup tile (zeroed; no DMA dependency) so dummy matmuls can
        # ramp the HAM clock-gate during the startup DMA wait
        warm_t = wpool.tile([128, RT], dt.bfloat16, tag="warm")
        nc.vector.memset(warm_t[:], 0.0)
        for wi in range(NWARM):
            wp_ = qpsum.tile([128, RT], dt.float32, tag="qp", name=f"warm{wi}")
            nc.tensor.matmul(wp_[:], lhsT=warm_t[:, 0:128], rhs=warm_t[:],
                             start=True, stop=True)

        # --- startup DMAs: rowtile 0's X^T + Wq lead the queues, spread
        # across the SP/ACT/GPSIMD/DVE trigger queues so they transfer in
        # parallel.  bf16 chunks (which start each Qproj chain) first.
        x0b = [x0pool.tile([128, RT], dt.bfloat16, tag=f"x0{k}", name=f"x0{k}")
               for k in range(NKB)]
        for k in range(NKB):
            nc.scalar.dma_start(out=x0b[k][:], in_=xtb[:, k, 0:RT])
        wqb_c = [wpool.tile([128, NKB * 128], dt.bfloat16, tag=f"wqb{cc}",
                            name=f"wqb{cc}") for cc in range(NK)]
        for cc in range(NK):
            nc.sync.dma_start(out=wqb_c[cc][:], in_=wqb[:, cc, :])
        x08 = x0pool.tile([128, 2, 2, RT], dt.float8e4, tag="x08")
        nc.gpsimd.dma_start(out=x08[:], in_=xt8[:, :, :, 0:RT])
        wq8_t = wpool.tile([128, NK, 2, 2, 128], dt.float8e4, tag="wq8")
        nc.gpsimd.dma_start(out=wq8_t[:], in_=wq8[:, :, :, :, :])

        kt_t = wpool.tile([128, H * 128], dt.bfloat16, tag="kt")
        nc.gpsimd.dma_start(out=kt_t[:], in_=kt[:, :])
        v64_t = wpool.tile([128, H * 128], dt.bfloat16, tag="v64")
        nc.gpsimd.dma_start(out=v64_t[:], in_=v64[:, :])
        sel_t = wpool.tile([128, H * 128], dt.bfloat16, tag="sel")
        nc.gpsimd.dma_start(out=sel_t[:], in_=sel[:, :])
        bias_t = wpool.tile([128, 1], dt.float32, tag="bias")
        nc.gpsimd.dma_start(out=bias_t[:], in_=biasr[:, :])
        weff_t = wpool.tile([128, NK * D], dt.bfloat16, tag="weff")
        nc.sync.dma_start(
            out=weff_t[:, :].rearrange("p (kc c) -> p kc c", c=D),
            in_=weff.rearrange("(kc p) c -> p kc c", p=128))

        prev = None
        for rt in range(NRT):
            # --- load X^T row-tile
            if rt > 0:
                xtb_t = xpool.tile([128, NKB * RT], dt.bfloat16, tag="xtb")
                nc.sync.dma_start(
                    out=xtb_t[:, :].rearrange("p (k r) -> p k r", r=RT),
                    in_=xtb[:, :, rt * RT:(rt + 1) * RT],
                )
                xt8_t = x8pool.tile([128, 2, 2, RT], dt.float8e4, tag="xt8")
                nc.scalar.dma_start(out=xt8_t[:],
                                    in_=xt8[:, :, :, rt * RT:(rt + 1) * RT])

            # --- Q^T = Wq^T @ X^T : 4 bf16 chunks + 2 fp8 DoubleRow pairs
            qt_t = qpool.tile([128, NK * RT], dt.bfloat16, tag="qt")
            for cc in range(NK):
                qp = qpsum.tile([128, RT], dt.float32, tag="qp")
                for k in range(NKB):
                    nc.tensor.matmul(
                        qp[:],
                        lhsT=wqb_c[cc][:, k * 128:(k + 1) * 128],
                        rhs=(x0b[k][:] if rt == 0
                             else xtb_t[:, k * RT:(k + 1) * RT]),
                        start=(k == 0), stop=False,
                    )
                for j in range(2):
                    nc.tensor.matmul(
                        qp[:],
                        lhsT=wq8_t[:, cc, j],
                        rhs=(x08[:, j] if rt == 0 else xt8_t[:, j]),
                        start=False, stop=(j == 1),
                        perf_mode=DR,
                    )
                nc.vector.tensor_copy(qt_t[:, cc * RT:(cc + 1) * RT], qp[:])

            # --- attention per head -> A^T packed [128, 8*512] (bf16)
            at_t = apool.tile([128, NK * RT], dt.bfloat16, tag="at")

            def wproj_gen(prt, pat, split_dma=False):
                # previous rowtile's Out projection as a stream of small
                # PE batches, pulled between attention heads so the PE has
                # dense independent work during ACT/DVE latencies.  PSUM
                # double-buffers by alternating the op pool with the (idle
                # during the head loop) Qproj pool.
                for rc4 in range(4):
                    ot = opool.tile([128, D], dt.float32, tag="ot")
                    for oc in range(2):
                        pool = opsum if (rc4 * 2 + oc) % 2 == 0 else qpsum
                        op_ = pool.tile([128, RT], dt.float32,
                                        tag="op" if pool is opsum else "qp")
                        for ac in range(NK):
                            nc.tensor.matmul(
                                op_[:],
                                lhsT=pat[:, ac * RT + rc4 * 128: ac * RT + (rc4 + 1) * 128],
                                rhs=weff_t[:, ac * D + oc * 512: ac * D + (oc + 1) * 512],
                                start=(ac == 0), stop=(ac == NK - 1),
                            )
                            if ac % 4 == 3:
                                yield
                        nc.vector.tensor_copy(ot[:, oc * 512:(oc + 1) * 512], op_[:])
                        if split_dma:
                            nc.sync.dma_start(
                                out=out[prt * RT + rc4 * 128: prt * RT + (rc4 + 1) * 128,
                                        oc * 512:(oc + 1) * 512],
                                in_=ot[:, oc * 512:(oc + 1) * 512],
                            )
                    if not split_dma:
                        nc.sync.dma_start(
                            out=out[prt * RT + rc4 * 128: prt * RT + (rc4 + 1) * 128, :],
                            in_=ot[:],
                        )

            wops = wproj_gen(prev[0], prev[1]) if prev is not None else None

            _done = object()

            def drain(k):
                if wops is None:
                    return
                for _ in range(k):
                    if next(wops, _done) is _done:
                        break

            au_list = []
            pair_ap = [None]
            rt_su = [None]

            def head_front(h):
                csl = bass.ds((h // 2) * RT, RT)
                sp = spsum.tile([128, RT], dt.float32, tag="sp")
                nc.tensor.matmul(
                    sp[:],
                    lhsT=kt_t[:, h * 128:(h + 1) * 128],
                    rhs=qt_t[:, csl],
                    start=True, stop=True,
                )
                # full-height exp: pad score rows are exact zeros from the
                # zero-padded qk weights, so pt rows 77..127 become exp(0)=1,
                # which the zero-padded sel/vp weight rows then ignore
                pt = ppool.tile([128, RT], dt.bfloat16, tag="pt")
                nc.scalar.activation(pt[:], sp[:],
                                     mybir.ActivationFunctionType.Exp,
                                     bias=bias_t[:])
                return pt

            def head_back(h, pt):
                po = (h % 2) * HD
                # head h's softmax sum -> row h of the shared [128, RT] tile
                # (rows 16..127 accumulate zeros from the padded selector)
                if h == 0:
                    rt_su[0] = supsum.tile([128, RT], dt.float32, tag="su",
                                           name="su")
                nc.tensor.matmul(
                    rt_su[0][:],
                    lhsT=sel_t[:, h * 128:(h + 1) * 128],
                    rhs=pt[:],
                    start=(h == 0), stop=(h == H - 1),
                )
                if po == 0:
                    ap_ = apsum.tile([128, RT], dt.float32, tag="ap")
                    pair_ap[0] = ap_
                nc.tensor.matmul(
                    pair_ap[0][:],
                    lhsT=v64_t[:, h * 128:(h + 1) * 128],
                    rhs=pt[:],
                    start=(po == 0), stop=(po != 0),
                )
                if po != 0:
                    # pair complete: stash unnormalized A^T pair in SBUF fp32
                    au = aupool.tile([128, RT], dt.float32, tag="au")
                    nc.vector.tensor_copy(au[:], pair_ap[0][:])
                    au_list.append(au)

            # software-pipelined head loop: exp(h) overlaps qk(h+1) and the
            # previous head's sums/PV plus a Wproj batch
            prev_pt = None
            for h in range(H):
                pt = head_front(h)
                if prev_pt is not None:
                    head_back(h - 1, prev_pt)
                prev_pt = pt
                if h >= 2:
                    drain(1)
            head_back(H - 1, prev_pt)

            # --- tail: one reciprocal for all 16 heads, bounced through DRAM
            # so broadcast-read DMAs can replicate each head's recip row to
            # its 64 partitions (gpsimd partition_broadcast is unreliable on
            # hw; DMA broadcast reads are exact)
            rcf = rcfpool.tile([16, RT], dt.float32, tag="rcf")
            nc.scalar.activation(rcf[0:16, :], rt_su[0][0:16, :],
                                 mybir.ActivationFunctionType.Ln)
            rcb = rcbpool.tile([16, RT], dt.bfloat16, tag="rcb")
            nc.scalar.activation(rcb[0:16, :], rcf[0:16, :],
                                 mybir.ActivationFunctionType.Exp,
                                 scale=-1.0)
            nc.scalar.dma_start(out=rcb_d[rt * 16:rt * 16 + 16, :],
                                in_=rcb[0:16, :])
            drain(2)
            for c in range(NK):
                rb = rbpool.tile([128, RT], dt.bfloat16, tag="rb")
                nc.gpsimd.dma_start(
                    out=rb[0:64, :],
                    in_=rcb_d[rt * 16 + 2 * c:rt * 16 + 2 * c + 1, :]
                    .to_broadcast((64, RT)))
                nc.gpsimd.dma_start(
                    out=rb[64:128, :],
                    in_=rcb_d[rt * 16 + 2 * c + 1:rt * 16 + 2 * c + 2, :]
                    .to_broadcast((64, RT)))
                nc.vector.tensor_mul(
                    at_t[:, c * RT:(c + 1) * RT], au_list[c][:], rb[:],
                )
                drain(1)
            drain(99)

            prev = (rt, at_t)

        # drain: Wproj of the final rowtile
        for _ in wproj_gen(prev[0], prev[1], split_dma=True):
            pass

    nc.compile()
    return nc


def _get_program():
    if "p" not in _PROG_CACHE:
        _PROG_CACHE["p"] = _build_program()
    return _PROG_CACHE["p"]


def _prep_inputs(x, te, mask, Wq, Wk, Wv, Wo, Wst):
    """Host-side fp32 weight prep + per-core shard maps."""
    K = (te @ Wk).reshape(B, TT, G, HD) * SCALE
    V = (te @ Wv).reshape(B, TT, G, HD)
    Weff = ((Wst[:D] + Wst[D:]) @ Wo).astype(np.float32)
    weff_b = Weff.astype(BF16)

    # fp8 half of Wq (chunks 0-3): wq8[p, cc, j, i, m] = Wq[(2j+i)*128+p, cc*128+m]
    wq8_b = np.ascontiguousarray(
        Wq[:512].reshape(2, 2, 128, NK, 128).transpose(2, 3, 0, 1, 4)
    ).astype(FP8)
    # bf16 half (chunks 4-7): wqb[p, cc, k*128+m] = Wq[(4+k)*128+p, cc*128+m]
    wqb_b = np.ascontiguousarray(
        Wq[512:].reshape(NKB, 128, NK, 128).transpose(1, 2, 0, 3).reshape(
            128, NK, NKB * 128)
    ).astype(BF16)

    # per-head one-hot sum selectors [128, H*128]: block h col h = 1 on rows 0..76
    sel_np = np.zeros((128, H * 128), np.float32)
    for h in range(H):
        sel_np[0:TT, h * 128 + h] = 1.0
    sel_b = sel_np.astype(BF16)

    kt_b, v_b, bias_b = [], [], []
    for b in range(B):
        # qk blocks [128, H*128]: block h rows (h%2)*64..+64, cols 0..76 = K_g^T
        ktq = np.zeros((128, H * 128), np.float32)
        # PV blocks [128, H*128]: block h rows 0..76, cols (h%2)*64..+64 = V_g
        vp = np.zeros((128, H * 128), np.float32)
        for h in range(H):
            g = h // HPG
            po = (h % 2) * HD
            ktq[po:po + HD, h * 128:h * 128 + TT] = K[b, :, g, :].T
            vp[0:TT, h * 128 + po:h * 128 + po + HD] = V[b, :, g, :]
        kt_b.append(ktq.astype(BF16))
        v_b.append(vp.astype(BF16))
        bv = np.zeros((128, 1), np.float32)
        bv[0:TT, 0] = np.where(mask[b], 0.0, -30.0)
        bias_b.append(bv)

    in_maps = []
    for c in range(NCORES):
        b = c // (NCORES // B)
        fr = (c % (NCORES // B)) * FPC
        xc = x[b, fr:fr + FPC].reshape(ROWS, D)
        # xt8[p, j, i, r] = X[r, (2j+i)*128+p] for chunks 0-3
        xt8_c = np.ascontiguousarray(
            xc[:, :512].T.reshape(2, 2, 128, ROWS)
            .transpose(2, 0, 1, 3)).astype(FP8)
        # xtb[p, k, r] = X[r, (4+k)*128+p]
        xtb_c = np.ascontiguousarray(
            xc[:, 512:].T.reshape(NKB, 128, ROWS).transpose(1, 0, 2)
        ).astype(BF16)
        m = {
            "xt8": xt8_c,
            "xtb": xtb_c,
            "wq8": wq8_b,
            "wqb": wqb_b,
            "weff": weff_b,
            "kt": kt_b[b],
            "v64": v_b[b],
            "sel": sel_b,
            "biasr": bias_b[b],
        }
        in_maps.append(m)
    return in_maps


def kernel(x, text_embeddings, padding_mask, use_mqa=0, use_qk_norm=0,
           Wq=None, Wk=None, Wv=None, Wo=None, Wst=None):
    global LAST_RESULTS
    x = np.asarray(x, np.float32)
    te = np.asarray(text_embeddings, np.float32)
    mask = np.asarray(padding_mask).astype(bool)
    Wq = np.asarray(Wq, np.float32)
    Wk = np.asarray(Wk, np.float32)
    Wv = np.asarray(Wv, np.float32)
    Wo = np.asarray(Wo, np.float32)
    Wst = np.asarray(Wst, np.float32)
    assert x.shape == (B, T, HW, D) and te.shape == (B, TT, D)

    in_maps = _prep_inputs(x, te, mask, Wq, Wk, Wv, Wo, Wst)
    nc = _get_program()

    res = run_bass_kernel_spmd(nc, in_maps, list(range(NCORES)),
                               trace=TRACE, **TRACE_KWARGS)
    LAST_RESULTS = res

    outp = np.empty((B, T, HW, D), np.float32)
    for c in range(NCORES):
        b = c // (NCORES // B)
        fr = (c % (NCORES // B)) * FPC
        outp[b, fr:fr + FPC] = res.results[c]["out"].reshape(FPC, HW, D)
    return outp


# revision 18
# speedup vs baseline: 1.0014x; 1.0014x over previous
"""Trainium2 Bass kernel for nn_FactorizedCrossAttention.

Key algebraic facts used (verified against the reference in fp64):
  * The "spatial" and "temporal" branches compute IDENTICAL per-position
    values: cross-attention over text tokens is independent per query row,
    and qt rows equal qs rows (same x row through the same Wq).  Hence
    spatial == temporal exactly.
  * concat([A, A]) @ Wst @ Wo == A @ ((Wst[:D] + Wst[D:]) @ Wo) — so both
    output projections fold into one 1024x1024 matrix Weff.
  * softmax scale (0.125) is folded into K on the host; the padding-mask
    bias is a per-token column vector applied inside the exp activation.

v2 changes vs the 345us baseline (PE-bound at 92.3%):
  * Qproj half-fp8: contraction chunks 0-3 run as 2 fp8e4 DoubleRow
    matmuls (2 k-subtiles each, 2x column rate), chunks 4-7 stay bf16;
    both accumulate into the same PSUM tile.  Empirical rel-err 0.0165
    (gate 2e-2) via exact numpy simulation of the quantization.
  * The per-rowtile reciprocal broadcast moved off the PE: rcb [16,512]
    is bounced through a DRAM scratch tensor and read back by 16
    broadcast-read DMAs (partition-replicating APs are only legal on the
    DRAM side; gpsimd partition_broadcast returns garbage on hw for most
    source/dest bases).  This removes the 8 bcast selector matmuls per
    rowtile from the PE.
  * PE warmup: dummy matmuls on a zeroed tile issue during the startup
    DMA wait so the HAM clock-gate ramps at ~11us instead of 19.5us.

PE per rowtile: Qproj 24576c + qk 8192 + sums 8192 + PV 8192 +
Wproj 32768 = 81920c (34.1us warm) vs 94208c baseline.

Sharding: pure data-parallel over (B, T_frames): 32 frames / 8 cores =
4 frames (4096 query rows) per core; K/V/weights replicated.  No
collectives.

Device layout is "transposed activations": X^T, Q^T, A^T all live as
[feature-part, row-free] tiles so every matmul is a natural slice.  Head h
occupies partitions (h%2)*64..+64 of feature chunk h//2; K^T is replicated
on both partition halves so odd heads read lane-aligned operands, and odd
heads' PV output is placed at PSUM base 64 (tile_position) so A^T lands on
partitions 64..127 without any cross-partition copies.
"""

import sys

if "/opt/trn_rl_repo" not in sys.path:
    sys.path.insert(0, "/opt/trn_rl_repo")

from contextlib import ExitStack

import ml_dtypes
import numpy as np

import concourse.bass as bass
import concourse.mybir as mybir
import concourse.tile as tile
from concourse import bacc
from concourse.bass_utils import run_bass_kernel_spmd

BF16 = ml_dtypes.bfloat16
FP8 = ml_dtypes.float8_e4m3

D = 1024           # d_model
H = 16             # num heads
G = 4              # query groups
HD = 64            # head dim
HPG = H // G       # heads per group
SCALE = 0.125
B, T, HW, TT = 2, 16, 1024, 77
NCORES = 8
FPC = (B * T) // NCORES      # frames per core = 4
ROWS = FPC * HW              # 4096 query rows per core
RT = 512                     # rows per row-tile
NRT = ROWS // RT             # 8
NK = D // 128                # 8 partition chunks of d_model
NKB = 4                      # bf16 contraction chunks (4..7)
NWARM = 12                   # PE warmup dummy matmuls

_PROG_CACHE = {}


def _patch_act_tables():
    """Force every activation onto the one table set that contains Exp, Ln
    and Copy together (natural_log_exp_and_others, same 400-interval
    precision).  Without this, bacc's table-load pass can alternate between
    table sets, costing a ~1.28us ACT_TABLE_LOAD per switch."""
    import concourse.bacc as _bm
    import concourse.hw_specs as _hw
    if getattr(_bm, "_act_tables_patched", False):
        return
    _orig = _hw.get_activation_tables

    def patched(arch):
        t = dict(_orig(arch))
        combo = None
        for name, funcs in t.items():
            if (mybir.ActivationFunctionType.Exp in funcs
                    and mybir.ActivationFunctionType.Ln in funcs):
                combo = name
                break
        if combo is not None:
            for name in list(t):
                if name != combo:
                    t[name] = set()
        return t

    _bm.get_activation_tables = patched
    _bm._act_tables_patched = True

# test.py can flip these for profiling runs
TRACE = False
TRACE_KWARGS = {}
LAST_RESULTS = None


def _build_program():
    _patch_act_tables()
    dt = mybir.dt
    DR = mybir.MatmulPerfMode.DoubleRow
    nc = bacc.Bacc("TRN2", target_bir_lowering=False, debug=False,
                   num_devices=NCORES)

    # fp8 half of X^T / Wq (contraction chunks 0-3 as 2 DoubleRow pairs)
    xt8 = nc.dram_tensor("xt8", [128, 2, 2, ROWS], dt.float8e4,
                         kind="ExternalInput").ap()
    wq8 = nc.dram_tensor("wq8", [128, NK, 2, 2, 128], dt.float8e4,
                         kind="ExternalInput").ap()
    # bf16 half (contraction chunks 4-7), chain-major
    xtb = nc.dram_tensor("xtb", [128, NKB, ROWS], dt.bfloat16,
                         kind="ExternalInput").ap()
    wqb = nc.dram_tensor("wqb", [128, NK, NKB * 128], dt.bfloat16,
                         kind="ExternalInput").ap()
    weff = nc.dram_tensor("weff", [D, D], dt.bfloat16, kind="ExternalInput").ap()
    # per-head qk blocks: [128, H*128], block h rows (h%2)*64..+64 = K_g^T
    kt = nc.dram_tensor("kt", [128, H * 128], dt.bfloat16, kind="ExternalInput").ap()
    # per-head PV blocks: [128, H*128], block h rows 0..76 = V_g at cols (h%2)*64..+64
    v64 = nc.dram_tensor("v64", [128, H * 128], dt.bfloat16, kind="ExternalInput").ap()
    # per-head one-hot sum selectors: [128, H*128], block h: col h = 1 on rows 0..76
    sel = nc.dram_tensor("sel", [128, H * 128], dt.bfloat16,
                         kind="ExternalInput").ap()
    # DRAM bounce buffer for the softmax reciprocals: row rt*16+h = head h's
    # recip for rowtile rt.  DMA reads replicate one row to 64 partitions
    # (broadcast-read APs are only legal on the DRAM side).
    rcb_d = nc.dram_tensor("rcb_d", [NRT * 16, RT], dt.bfloat16,
                           kind="Internal").ap()
    # padding-mask bias as a per-partition column vector, applied inside the
    # exp activation (func(scale*in + bias)); all-zero when the mask is all-True
    biasr = nc.dram_tensor("biasr", [128, 1], dt.float32, kind="ExternalInput").ap()
    out = nc.dram_tensor("out", [ROWS, D], dt.float32, kind="ExternalOutput").ap()

    with tile.TileContext(nc) as tc, ExitStack() as ctx:
        wpool = ctx.enter_context(tc.tile_pool(name="weights", bufs=1))
        xpool = ctx.enter_context(tc.tile_pool(name="xt", bufs=3))
        x8pool = ctx.enter_context(tc.tile_pool(name="xt8", bufs=3))
        x0pool = ctx.enter_context(tc.tile_pool(name="xt0", bufs=1))
        qpool = ctx.enter_context(tc.tile_pool(name="qt", bufs=2))
        apool = ctx.enter_context(tc.tile_pool(name="at", bufs=2))
        ppool = ctx.enter_context(tc.tile_pool(name="pt", bufs=4))
        aupool = ctx.enter_context(tc.tile_pool(name="au", bufs=9))
        rcfpool = ctx.enter_context(tc.tile_pool(name="rcf", bufs=2))
        rcbpool = ctx.enter_context(tc.tile_pool(name="rcb", bufs=2))
        rbpool = ctx.enter_context(tc.tile_pool(name="rb", bufs=9))
        opool = ctx.enter_context(tc.tile_pool(name="osb", bufs=3))
        # 8 PSUM banks: qp(2) + sp(2) + ap(2) + su(1) + op(1)
        qpsum = ctx.enter_context(tc.tile_pool(name="qpsum", bufs=2, space="PSUM"))
        spsum = ctx.enter_context(tc.tile_pool(name="spsum", bufs=2, space="PSUM"))
        apsum = ctx.enter_context(tc.tile_pool(name="apsum", bufs=2, space="PSUM"))
        supsum = ctx.enter_context(tc.tile_pool(name="supsum", bufs=1, space="PSUM"))
        opsum = ctx.enter_context(tc.tile_pool(name="opsum", bufs=1, space="PSUM"))

        # --- PE warmup tile (zeroed; no DMA dependency) so dummy matmuls can
        # ramp the HAM clock-gate during the startup DMA wait
        warm_t = wpool.tile([128, RT], dt.bfloat16, tag="warm")
        nc.vector.memset(warm_t[:], 0.0)
        for wi in range(NWARM):
            wp_ = qpsum.tile([128, RT], dt.float32, tag="qp", name=f"warm{wi}")
            nc.tensor.matmul(wp_[:], lhsT=warm_t[:, 0:128], rhs=warm_t[:],
                             start=True, stop=True)

        # --- startup DMAs: rowtile 0's X^T + Wq lead the queues, spread
        # across the SP/ACT/GPSIMD/DVE trigger queues so they transfer in
        # parallel.  bf16 chunks (which start each Qproj chain) first.
        x0b = [x0pool.tile([128, RT], dt.bfloat16, tag=f"x0{k}", name=f"x0{k}")
               for k in range(NKB)]
        for k in range(NKB):
            nc.scalar.dma_start(out=x0b[k][:], in_=xtb[:, k, 0:RT])
        wqb_c = [wpool.tile([128, NKB * 128], dt.bfloat16, tag=f"wqb{cc}",
                            name=f"wqb{cc}") for cc in range(NK)]
        for cc in range(NK):
            nc.sync.dma_start(out=wqb_c[cc][:], in_=wqb[:, cc, :])
        x08 = x0pool.tile([128, 2, 2, RT], dt.float8e4, tag="x08")
        nc.gpsimd.dma_start(out=x08[:], in_=xt8[:, :, :, 0:RT])
        wq8_t = wpool.tile([128, NK, 2, 2, 128], dt.float8e4, tag="wq8")
        nc.gpsimd.dma_start(out=wq8_t[:], in_=wq8[:, :, :, :, :])

        kt_t = wpool.tile([128, H * 128], dt.bfloat16, tag="kt")
        nc.gpsimd.dma_start(out=kt_t[:], in_=kt[:, :])
        v64_t = wpool.tile([128, H * 128], dt.bfloat16, tag="v64")
        nc.gpsimd.dma_start(out=v64_t[:], in_=v64[:, :])
        sel_t = wpool.tile([128, H * 128], dt.bfloat16, tag="sel")
        nc.gpsimd.dma_start(out=sel_t[:], in_=sel[:, :])
        bias_t = wpool.tile([128, 1], dt.float32, tag="bias")
        nc.gpsimd.dma_start(out=bias_t[:], in_=biasr[:, :])
        weff_t = wpool.tile([128, NK * D], dt.bfloat16, tag="weff")
        nc.sync.dma_start(
            out=weff_t[:, :].rearrange("p (kc c) -> p kc c", c=D),
            in_=weff.rearrange("(kc p) c -> p kc c", p=128))

        prev = None
        pending_muls = []
        for rt in range(NRT):
            # --- load X^T row-tile
            if rt > 0:
                xtb_t = xpool.tile([128, NKB * RT], dt.bfloat16, tag="xtb")
                nc.sync.dma_start(
                    out=xtb_t[:, :].rearrange("p (k r) -> p k r", r=RT),
                    in_=xtb[:, :, rt * RT:(rt + 1) * RT],
                )
                xt8_t = x8pool.tile([128, 2, 2, RT], dt.float8e4, tag="xt8")
                nc.scalar.dma_start(out=xt8_t[:],
                                    in_=xt8[:, :, :, rt * RT:(rt + 1) * RT])

            # --- Q^T = Wq^T @ X^T : 4 bf16 chunks + 2 fp8 DoubleRow pairs
            qt_t = qpool.tile([128, NK * RT], dt.bfloat16, tag="qt")
            for cc in range(NK):
                qp = qpsum.tile([128, RT], dt.float32, tag="qp")
                for k in range(NKB):
                    nc.tensor.matmul(
                        qp[:],
                        lhsT=wqb_c[cc][:, k * 128:(k + 1) * 128],
                        rhs=(x0b[k][:] if rt == 0
                             else xtb_t[:, k * RT:(k + 1) * RT]),
                        start=(k == 0), stop=False,
                    )
                for j in range(2):
                    nc.tensor.matmul(
                        qp[:],
                        lhsT=wq8_t[:, cc, j],
                        rhs=(x08[:, j] if rt == 0 else xt8_t[:, j]),
                        start=False, stop=(j == 1),
                        perf_mode=DR,
                    )
                nc.vector.tensor_copy(qt_t[:, cc * RT:(cc + 1) * RT], qp[:])

            # --- deferred normalization muls of the PREVIOUS rowtile: issued
            # after this rowtile's qt copies so the DVE FIFO isn't clogged by
            # muls stuck behind the DMA-bounce latency (which would
            # head-of-line-block the qt copies and stall the PE ~2.5us/rt)
            for pat, pau, prb, pc in pending_muls:
                nc.vector.tensor_mul(
                    pat[:, pc * RT:(pc + 1) * RT], pau[:], prb[:],
                )
            pending_muls = []

            # --- attention per head -> A^T packed [128, 8*512] (bf16)
            at_t = apool.tile([128, NK * RT], dt.bfloat16, tag="at")

            def wproj_gen(prt, pat, split_dma=False):
                # previous rowtile's Out projection as a stream of small
                # PE batches, pulled between attention heads so the PE has
                # dense independent work during ACT/DVE latencies.  PSUM
                # double-buffers by alternating the op pool with the (idle
                # during the head loop) Qproj pool.
                for rc4 in range(4):
                    ot = opool.tile([128, D], dt.float32, tag="ot")
                    for oc in range(2):
                        pool = opsum if (rc4 * 2 + oc) % 2 == 0 else qpsum
                        op_ = pool.tile([128, RT], dt.float32,
                                        tag="op" if pool is opsum else "qp")
                        for ac in range(NK):
                            nc.tensor.matmul(
                                op_[:],
                                lhsT=pat[:, ac * RT + rc4 * 128: ac * RT + (rc4 + 1) * 128],
                                rhs=weff_t[:, ac * D + oc * 512: ac * D + (oc + 1) * 512],
                                start=(ac == 0), stop=(ac == NK - 1),
                            )
                            if ac % 4 == 3:
                                yield
                        nc.vector.tensor_copy(ot[:, oc * 512:(oc + 1) * 512], op_[:])
                        if split_dma:
                            nc.sync.dma_start(
                                out=out[prt * RT + rc4 * 128: prt * RT + (rc4 + 1) * 128,
                                        oc * 512:(oc + 1) * 512],
                                in_=ot[:, oc * 512:(oc + 1) * 512],
                            )
                    if not split_dma:
                        nc.sync.dma_start(
                            out=out[prt * RT + rc4 * 128: prt * RT + (rc4 + 1) * 128, :],
                            in_=ot[:],
                        )

            wops = wproj_gen(prev[0], prev[1]) if prev is not None else None

            _done = object()

            def drain(k):
                if wops is None:
                    return
                for _ in range(k):
                    if next(wops, _done) is _done:
                        break

            au_list = []
            pair_ap = [None]
            rt_su = [None]

            def head_front(h):
                csl = bass.ds((h // 2) * RT, RT)
                sp = spsum.tile([128, RT], dt.float32, tag="sp")
                nc.tensor.matmul(
                    sp[:],
                    lhsT=kt_t[:, h * 128:(h + 1) * 128],
                    rhs=qt_t[:, csl],
                    start=True, stop=True,
                )
                # full-height exp: pad score rows are exact zeros from the
                # zero-padded qk weights, so pt rows 77..127 become exp(0)=1,
                # which the zero-padded sel/vp weight rows then ignore
                pt = ppool.tile([128, RT], dt.bfloat16, tag="pt")
                nc.scalar.activation(pt[:], sp[:],
                                     mybir.ActivationFunctionType.Exp,
                                     bias=bias_t[:])
                return pt

            def head_back(h, pt):
                po = (h % 2) * HD
                # head h's softmax sum -> row h of the shared [128, RT] tile
                # (rows 16..127 accumulate zeros from the padded selector)
                if h == 0:
                    rt_su[0] = supsum.tile([128, RT], dt.float32, tag="su",
                                           name="su")
                nc.tensor.matmul(
                    rt_su[0][:],
                    lhsT=sel_t[:, h * 128:(h + 1) * 128],
                    rhs=pt[:],
                    start=(h == 0), stop=(h == H - 1),
                )
                if po == 0:
                    ap_ = apsum.tile([128, RT], dt.float32, tag="ap")
                    pair_ap[0] = ap_
                nc.tensor.matmul(
                    pair_ap[0][:],
                    lhsT=v64_t[:, h * 128:(h + 1) * 128],
                    rhs=pt[:],
                    start=(po == 0), stop=(po != 0),
                )
                if po != 0:
                    # pair complete: stash unnormalized A^T pair in SBUF fp32
                    au = aupool.tile([128, RT], dt.float32, tag="au")
                    nc.vector.tensor_copy(au[:], pair_ap[0][:])
                    au_list.append(au)

            # software-pipelined head loop: exp(h) overlaps qk(h+1) and the
            # previous head's sums/PV plus a Wproj batch
            prev_pt = None
            for h in range(H):
                pt = head_front(h)
                if prev_pt is not None:
                    head_back(h - 1, prev_pt)
                prev_pt = pt
                if h >= 2:
                    drain(1)
            head_back(H - 1, prev_pt)

            # --- tail: one reciprocal for all 16 heads, bounced through DRAM
            # so broadcast-read DMAs can replicate each head's recip row to
            # its 64 partitions (gpsimd partition_broadcast is unreliable on
            # hw; DMA broadcast reads are exact)
            rcf = rcfpool.tile([16, RT], dt.float32, tag="rcf")
            nc.scalar.activation(rcf[0:16, :], rt_su[0][0:16, :],
                                 mybir.ActivationFunctionType.Ln)
            rcb = rcbpool.tile([16, RT], dt.bfloat16, tag="rcb")
            nc.scalar.activation(rcb[0:16, :], rcf[0:16, :],
                                 mybir.ActivationFunctionType.Exp,
                                 scale=-1.0)
            nc.scalar.dma_start(out=rcb_d[rt * 16:rt * 16 + 16, :],
                                in_=rcb[0:16, :])
            for c in range(NK):
                rb = rbpool.tile([128, RT], dt.bfloat16, tag="rb")
                nc.gpsimd.dma_start(
                    out=rb[0:64, :],
                    in_=rcb_d[rt * 16 + 2 * c:rt * 16 + 2 * c + 1, :]
                    .to_broadcast((64, RT)))
                nc.gpsimd.dma_start(
                    out=rb[64:128, :],
                    in_=rcb_d[rt * 16 + 2 * c + 1:rt * 16 + 2 * c + 2, :]
                    .to_broadcast((64, RT)))
                if rt < NRT - 1:
                    pending_muls.append((at_t, au_list[c], rb, c))
                else:
                    nc.vector.tensor_mul(
                        at_t[:, c * RT:(c + 1) * RT], au_list[c][:], rb[:],
                    )
                drain(1)
            drain(99)

            prev = (rt, at_t)

        # drain: Wproj of the final rowtile
        for _ in wproj_gen(prev[0], prev[1], split_dma=True):
            pass

    nc.compile()
    return nc


def _get_program():
    if "p" not in _PROG_CACHE:
        _PROG_CACHE["p"] = _build_program()
    return _PROG_CACHE["p"]


def _prep_inputs(x, te, mask, Wq, Wk, Wv, Wo, Wst):
    """Host-side fp32 weight prep + per-core shard maps."""
    K = (te @ Wk).reshape(B, TT, G, HD) * SCALE
    V = (te @ Wv).reshape(B, TT, G, HD)
    Weff = ((Wst[:D] + Wst[D:]) @ Wo).astype(np.float32)
    weff_b = Weff.astype(BF16)

    # fp8 half of Wq (chunks 0-3): wq8[p, cc, j, i, m] = Wq[(2j+i)*128+p, cc*128+m]
    wq8_b = np.ascontiguousarray(
        Wq[:512].reshape(2, 2, 128, NK, 128).transpose(2, 3, 0, 1, 4)
    ).astype(FP8)
    # bf16 half (chunks 4-7): wqb[p, cc, k*128+m] = Wq[(4+k)*128+p, cc*128+m]
    wqb_b = np.ascontiguousarray(
        Wq[512:].reshape(NKB, 128, NK, 128).transpose(1, 2, 0, 3).reshape(
            128, NK, NKB * 128)
    ).astype(BF16)

    # per-head one-hot sum selectors [128, H*128]: block h col h = 1 on rows 0..76
    sel_np = np.zeros((128, H * 128), np.float32)
    for h in range(H):
        sel_np[0:TT, h * 128 + h] = 1.0
    sel_b = sel_np.astype(BF16)

    kt_b, v_b, bias_b = [], [], []
    for b in range(B):
        # qk blocks [128, H*128]: block h rows (h%2)*64..+64, cols 0..76 = K_g^T
        ktq = np.zeros((128, H * 128), np.float32)
        # PV blocks [128, H*128]: block h rows 0..76, cols (h%2)*64..+64 = V_g
        vp = np.zeros((128, H * 128), np.float32)
        for h in range(H):
            g = h // HPG
            po = (h % 2) * HD
            ktq[po:po + HD, h * 128:h * 128 + TT] = K[b, :, g, :].T
            vp[0:TT, h * 128 + po:h * 128 + po + HD] = V[b, :, g, :]
        kt_b.append(ktq.astype(BF16))
        v_b.append(vp.astype(BF16))
        bv = np.zeros((128, 1), np.float32)
        bv[0:TT, 0] = np.where(mask[b], 0.0, -30.0)
        bias_b.append(bv)

    in_maps = []
    for c in range(NCORES):
        b = c // (NCORES // B)
        fr = (c % (NCORES // B)) * FPC
        xc = x[b, fr:fr + FPC].reshape(ROWS, D)
        # xt8[p, j, i, r] = X[r, (2j+i)*128+p] for chunks 0-3
        xt8_c = np.ascontiguousarray(
            xc[:, :512].T.reshape(2, 2, 128, ROWS)
            .transpose(2, 0, 1, 3)).astype(FP8)
        # xtb[p, k, r] = X[r, (4+k)*128+p]
        xtb_c = np.ascontiguousarray(
            xc[:, 512:].T.reshape(NKB, 128, ROWS).transpose(1, 0, 2)
        ).astype(BF16)
        m = {
            "xt8": xt8_c,
            "xtb": xtb_c,
            "wq8": wq8_b,
            "wqb": wqb_b,
            "weff": weff_b,
            "kt": kt_b[b],
            "v64": v_b[b],
            "sel": sel_b,
            "biasr": bias_b[b],
        }
        in_maps.append(m)
    return in_maps


def kernel(x, text_embeddings, padding_mask, use_mqa=0, use_qk_norm=0,
           Wq=None, Wk=None, Wv=None, Wo=None, Wst=None):
    global LAST_RESULTS
    x = np.asarray(x, np.float32)
    te = np.asarray(text_embeddings, np.float32)
    mask = np.asarray(padding_mask).astype(bool)
    Wq = np.asarray(Wq, np.float32)
    Wk = np.asarray(Wk, np.float32)
    Wv = np.asarray(Wv, np.float32)
    Wo = np.asarray(Wo, np.float32)
    Wst = np.asarray(Wst, np.float32)
    assert x.shape == (B, T, HW, D) and te.shape == (B, TT, D)

    in_maps = _prep_inputs(x, te, mask, Wq, Wk, Wv, Wo, Wst)
    nc = _get_program()

    res = run_bass_kernel_spmd(nc, in_maps, list(range(NCORES)),
                               trace=TRACE, **TRACE_KWARGS)
    LAST_RESULTS = res

    outp = np.empty((B, T, HW, D), np.float32)
    for c in range(NCORES):
        b = c // (NCORES // B)
        fr = (c % (NCORES // B)) * FPC
        outp[b, fr:fr + FPC] = res.results[c]["out"].reshape(FPC, HW, D)
    return outp


# revision 21
# speedup vs baseline: 1.0954x; 1.0938x over previous
"""Trainium2 Bass kernel for nn_FactorizedCrossAttention.

Key algebraic facts used (verified against the reference in fp64):
  * The "spatial" and "temporal" branches compute IDENTICAL per-position
    values: cross-attention over text tokens is independent per query row,
    and qt rows equal qs rows (same x row through the same Wq).  Hence
    spatial == temporal exactly.
  * concat([A, A]) @ Wst @ Wo == A @ ((Wst[:D] + Wst[D:]) @ Wo) — so both
    output projections fold into one 1024x1024 matrix Weff.
  * softmax scale (0.125) is folded into K on the host; the padding-mask
    bias is a per-token column vector applied inside the exp activation.

v2 changes vs the 345us baseline (PE-bound at 92.3%):
  * Qproj half-fp8: contraction chunks 0-3 run as 2 fp8e4 DoubleRow
    matmuls (2 k-subtiles each, 2x column rate), chunks 4-7 stay bf16;
    both accumulate into the same PSUM tile.  Empirical rel-err 0.0165
    (gate 2e-2) via exact numpy simulation of the quantization.
  * The per-rowtile reciprocal broadcast moved off the PE: rcb [16,512]
    is bounced through a DRAM scratch tensor and read back by 16
    broadcast-read DMAs (partition-replicating APs are only legal on the
    DRAM side; gpsimd partition_broadcast returns garbage on hw for most
    source/dest bases).  This removes the 8 bcast selector matmuls per
    rowtile from the PE.
  * PE warmup: dummy matmuls on a zeroed tile issue during the startup
    DMA wait so the HAM clock-gate ramps at ~11us instead of 19.5us.

PE per rowtile: Qproj 24576c + qk 8192 + sums 8192 + PV 8192 +
Wproj 32768 = 81920c (34.1us warm) vs 94208c baseline.

Sharding: pure data-parallel over (B, T_frames): 32 frames / 8 cores =
4 frames (4096 query rows) per core; K/V/weights replicated.  No
collectives.

Device layout is "transposed activations": X^T, Q^T, A^T all live as
[feature-part, row-free] tiles so every matmul is a natural slice.  Head h
occupies partitions (h%2)*64..+64 of feature chunk h//2; K^T is replicated
on both partition halves so odd heads read lane-aligned operands, and odd
heads' PV output is placed at PSUM base 64 (tile_position) so A^T lands on
partitions 64..127 without any cross-partition copies.
"""

import sys

if "/opt/trn_rl_repo" not in sys.path:
    sys.path.insert(0, "/opt/trn_rl_repo")

from contextlib import ExitStack

import ml_dtypes
import numpy as np

import concourse.bass as bass
import concourse.mybir as mybir
import concourse.tile as tile
from concourse import bacc
from concourse.bass_utils import run_bass_kernel_spmd

BF16 = ml_dtypes.bfloat16
FP8 = ml_dtypes.float8_e4m3

D = 1024           # d_model
H = 16             # num heads
G = 4              # query groups
HD = 64            # head dim
HPG = H // G       # heads per group
SCALE = 0.125
B, T, HW, TT = 2, 16, 1024, 77
NCORES = 8
FPC = (B * T) // NCORES      # frames per core = 4
ROWS = FPC * HW              # 4096 query rows per core
RT = 512                     # rows per row-tile
NRT = ROWS // RT             # 8
NK = D // 128                # 8 partition chunks of d_model
NKB = 4                      # bf16 contraction chunks (4..7)
NWARM = 16                   # PE warmup dummy matmuls

_PROG_CACHE = {}


def _patch_act_tables():
    """Force every activation onto the one table set that contains Exp, Ln
    and Copy together (natural_log_exp_and_others, same 400-interval
    precision).  Without this, bacc's table-load pass can alternate between
    table sets, costing a ~1.28us ACT_TABLE_LOAD per switch."""
    import concourse.bacc as _bm
    import concourse.hw_specs as _hw
    if getattr(_bm, "_act_tables_patched", False):
        return
    _orig = _hw.get_activation_tables

    def patched(arch):
        t = dict(_orig(arch))
        combo = None
        for name, funcs in t.items():
            if (mybir.ActivationFunctionType.Exp in funcs
                    and mybir.ActivationFunctionType.Ln in funcs):
                combo = name
                break
        if combo is not None:
            for name in list(t):
                if name != combo:
                    t[name] = set()
        return t

    _bm.get_activation_tables = patched
    _bm._act_tables_patched = True

# test.py can flip these for profiling runs
TRACE = False
TRACE_KWARGS = {}
LAST_RESULTS = None


def _build_program():
    _patch_act_tables()
    dt = mybir.dt
    DR = mybir.MatmulPerfMode.DoubleRow
    nc = bacc.Bacc("TRN2", target_bir_lowering=False, debug=False,
                   num_devices=NCORES)

    # fp8 half of X^T / Wq (contraction chunks 0-3 as 2 DoubleRow pairs)
    xt8 = nc.dram_tensor("xt8", [128, 2, 2, ROWS], dt.float8e4,
                         kind="ExternalInput").ap()
    wq8 = nc.dram_tensor("wq8", [128, NK, 2, 2, 128], dt.float8e4,
                         kind="ExternalInput").ap()
    # bf16 half (contraction chunks 4-7), chain-major
    xtb = nc.dram_tensor("xtb", [128, NKB, ROWS], dt.bfloat16,
                         kind="ExternalInput").ap()
    wqb = nc.dram_tensor("wqb", [128, NK, NKB * 128], dt.bfloat16,
                         kind="ExternalInput").ap()
    weff = nc.dram_tensor("weff", [D, D], dt.bfloat16, kind="ExternalInput").ap()
    # per-head qk blocks: [128, H*128], block h rows (h%2)*64..+64 = K_g^T
    kt = nc.dram_tensor("kt", [128, H * 128], dt.bfloat16, kind="ExternalInput").ap()
    # per-head PV blocks: [128, H*128], block h rows 0..76 = V_g at cols (h%2)*64..+64
    v64 = nc.dram_tensor("v64", [128, H * 128], dt.bfloat16, kind="ExternalInput").ap()
    # per-head one-hot sum selectors: [128, H*128], block h: col h = 1 on rows 0..76
    sel = nc.dram_tensor("sel", [128, H * 128], dt.bfloat16,
                         kind="ExternalInput").ap()
    # DRAM bounce buffer for the softmax reciprocals: row rt*16+h = head h's
    # recip for rowtile rt.  DMA reads replicate one row to 64 partitions
    # (broadcast-read APs are only legal on the DRAM side).
    rcb_d = nc.dram_tensor("rcb_d", [NRT * 16, RT], dt.bfloat16,
                           kind="Internal").ap()
    # padding-mask bias as a per-partition column vector, applied inside the
    # exp activation (func(scale*in + bias)); all-zero when the mask is all-True
    biasr = nc.dram_tensor("biasr", [128, 1], dt.float32, kind="ExternalInput").ap()
    out = nc.dram_tensor("out", [ROWS, D], dt.float32, kind="ExternalOutput").ap()

    with tile.TileContext(nc) as tc, ExitStack() as ctx:
        wpool = ctx.enter_context(tc.tile_pool(name="weights", bufs=1))
        xpool = ctx.enter_context(tc.tile_pool(name="xt", bufs=3))
        x8pool = ctx.enter_context(tc.tile_pool(name="xt8", bufs=3))
        x0pool = ctx.enter_context(tc.tile_pool(name="xt0", bufs=1))
        qpool = ctx.enter_context(tc.tile_pool(name="qt", bufs=2))
        apool = ctx.enter_context(tc.tile_pool(name="at", bufs=2))
        ppool = ctx.enter_context(tc.tile_pool(name="pt", bufs=4))
        aupool = ctx.enter_context(tc.tile_pool(name="au", bufs=9))
        rcfpool = ctx.enter_context(tc.tile_pool(name="rcf", bufs=2))
        rcbpool = ctx.enter_context(tc.tile_pool(name="rcb", bufs=2))
        rbpool = ctx.enter_context(tc.tile_pool(name="rb", bufs=9))
        opool = ctx.enter_context(tc.tile_pool(name="osb", bufs=3))
        # 8 PSUM banks: qp(2) + sp(2) + ap(2) + su(1) + op(1)
        qpsum = ctx.enter_context(tc.tile_pool(name="qpsum", bufs=2, space="PSUM"))
        spsum = ctx.enter_context(tc.tile_pool(name="spsum", bufs=2, space="PSUM"))
        apsum = ctx.enter_context(tc.tile_pool(name="apsum", bufs=2, space="PSUM"))
        supsum = ctx.enter_context(tc.tile_pool(name="supsum", bufs=1, space="PSUM"))
        opsum = ctx.enter_context(tc.tile_pool(name="opsum", bufs=1, space="PSUM"))

        # --- PE warmup tile (zeroed; no DMA dependency) so dummy matmuls can
        # ramp the HAM clock-gate during the startup DMA wait
        warm_t = wpool.tile([128, RT], dt.bfloat16, tag="warm")
        nc.vector.memset(warm_t[:], 0.0)
        for wi in range(NWARM):
            wp_ = qpsum.tile([128, RT], dt.float32, tag="qp", name=f"warm{wi}")
            nc.tensor.matmul(wp_[:], lhsT=warm_t[:, 0:128], rhs=warm_t[:],
                             start=True, stop=True)

        # --- startup DMAs: rowtile 0's X^T + Wq lead the queues, spread
        # across the SP/ACT/GPSIMD/DVE trigger queues so they transfer in
        # parallel.  bf16 chunks (which start each Qproj chain) first.
        x0b = [x0pool.tile([128, RT], dt.bfloat16, tag=f"x0{k}", name=f"x0{k}")
               for k in range(NKB)]
        for k in range(NKB):
            nc.scalar.dma_start(out=x0b[k][:], in_=xtb[:, k, 0:RT])
        wqb_c = [wpool.tile([128, NKB * 128], dt.bfloat16, tag=f"wqb{cc}",
                            name=f"wqb{cc}") for cc in range(NK)]
        for cc in range(NK):
            nc.sync.dma_start(out=wqb_c[cc][:], in_=wqb[:, cc, :])
        x08 = x0pool.tile([128, 2, 2, RT], dt.float8e4, tag="x08")
        nc.gpsimd.dma_start(out=x08[:], in_=xt8[:, :, :, 0:RT])
        wq8_t = wpool.tile([128, NK, 2, 2, 128], dt.float8e4, tag="wq8")
        nc.gpsimd.dma_start(out=wq8_t[:], in_=wq8[:, :, :, :, :])

        kt_t = wpool.tile([128, H * 128], dt.bfloat16, tag="kt")
        nc.gpsimd.dma_start(out=kt_t[:], in_=kt[:, :])
        v64_t = wpool.tile([128, H * 128], dt.bfloat16, tag="v64")
        nc.gpsimd.dma_start(out=v64_t[:], in_=v64[:, :])
        sel_t = wpool.tile([128, H * 128], dt.bfloat16, tag="sel")
        nc.gpsimd.dma_start(out=sel_t[:], in_=sel[:, :])
        bias_t = wpool.tile([128, 1], dt.float32, tag="bias")
        nc.gpsimd.dma_start(out=bias_t[:], in_=biasr[:, :])
        weff_t = wpool.tile([128, NK * D], dt.bfloat16, tag="weff")
        nc.sync.dma_start(
            out=weff_t[:, :].rearrange("p (kc c) -> p kc c", c=D),
            in_=weff.rearrange("(kc p) c -> p kc c", p=128))

        prev = None
        pending_muls = []
        for rt in range(NRT):
            # --- load X^T row-tile
            if rt > 0:
                xtb_t = xpool.tile([128, NKB * RT], dt.bfloat16, tag="xtb")
                nc.sync.dma_start(
                    out=xtb_t[:, :].rearrange("p (k r) -> p k r", r=RT),
                    in_=xtb[:, :, rt * RT:(rt + 1) * RT],
                )
                xt8_t = x8pool.tile([128, 2, 2, RT], dt.float8e4, tag="xt8")
                nc.scalar.dma_start(out=xt8_t[:],
                                    in_=xt8[:, :, :, rt * RT:(rt + 1) * RT])

            # --- Q^T = Wq^T @ X^T : 4 bf16 chunks + 2 fp8 DoubleRow pairs
            qt_t = qpool.tile([128, NK * RT], dt.bfloat16, tag="qt")
            for cc in range(NK):
                qp = qpsum.tile([128, RT], dt.float32, tag="qp")
                for k in range(NKB):
                    nc.tensor.matmul(
                        qp[:],
                        lhsT=wqb_c[cc][:, k * 128:(k + 1) * 128],
                        rhs=(x0b[k][:] if rt == 0
                             else xtb_t[:, k * RT:(k + 1) * RT]),
                        start=(k == 0), stop=False,
                    )
                for j in range(2):
                    nc.tensor.matmul(
                        qp[:],
                        lhsT=wq8_t[:, cc, j],
                        rhs=(x08[:, j] if rt == 0 else xt8_t[:, j]),
                        start=False, stop=(j == 1),
                        perf_mode=DR,
                    )
                nc.vector.tensor_copy(qt_t[:, cc * RT:(cc + 1) * RT], qp[:])

            # --- deferred normalization muls of the PREVIOUS rowtile: issued
            # after this rowtile's qt copies so the DVE FIFO isn't clogged by
            # muls stuck behind the DMA-bounce latency (which would
            # head-of-line-block the qt copies and stall the PE ~2.5us/rt)
            for pat, pau, prb, pc in pending_muls:
                nc.vector.tensor_mul(
                    pat[:, pc * RT:(pc + 1) * RT], pau[:], prb[:],
                )
            pending_muls = []

            # --- attention per head -> A^T packed [128, 8*512] (bf16)
            at_t = apool.tile([128, NK * RT], dt.bfloat16, tag="at")

            def wproj_gen(prt, pat, split_dma=False):
                # previous rowtile's Out projection as a stream of small
                # PE batches, pulled between attention heads so the PE has
                # dense independent work during ACT/DVE latencies.  PSUM
                # double-buffers by alternating the op pool with the (idle
                # during the head loop) Qproj pool.
                for rc4 in range(4):
                    ot = opool.tile([128, D], dt.float32, tag="ot")
                    for oc in range(2):
                        pool = opsum if (rc4 * 2 + oc) % 2 == 0 else qpsum
                        op_ = pool.tile([128, RT], dt.float32,
                                        tag="op" if pool is opsum else "qp")
                        for ac in range(NK):
                            nc.tensor.matmul(
                                op_[:],
                                lhsT=pat[:, ac * RT + rc4 * 128: ac * RT + (rc4 + 1) * 128],
                                rhs=weff_t[:, ac * D + oc * 512: ac * D + (oc + 1) * 512],
                                start=(ac == 0), stop=(ac == NK - 1),
                            )
                            if ac % 4 == 3:
                                yield
                        nc.vector.tensor_copy(ot[:, oc * 512:(oc + 1) * 512], op_[:])
                        if split_dma:
                            nc.sync.dma_start(
                                out=out[prt * RT + rc4 * 128: prt * RT + (rc4 + 1) * 128,
                                        oc * 512:(oc + 1) * 512],
                                in_=ot[:, oc * 512:(oc + 1) * 512],
                            )
                    if not split_dma:
                        nc.sync.dma_start(
                            out=out[prt * RT + rc4 * 128: prt * RT + (rc4 + 1) * 128, :],
                            in_=ot[:],
                        )

            wops = wproj_gen(prev[0], prev[1]) if prev is not None else None

            _done = object()

            def drain(k):
                if wops is None:
                    return
                for _ in range(k):
                    if next(wops, _done) is _done:
                        break

            au_list = []
            pair_ap = [None]
            rt_su = [None]

            def head_front(h):
                csl = bass.ds((h // 2) * RT, RT)
                sp = spsum.tile([128, RT], dt.float32, tag="sp")
                nc.tensor.matmul(
                    sp[:],
                    lhsT=kt_t[:, h * 128:(h + 1) * 128],
                    rhs=qt_t[:, csl],
                    start=True, stop=True,
                )
                # full-height exp: pad score rows are exact zeros from the
                # zero-padded qk weights, so pt rows 77..127 become exp(0)=1,
                # which the zero-padded sel/vp weight rows then ignore
                pt = ppool.tile([128, RT], dt.bfloat16, tag="pt")
                nc.scalar.activation(pt[:], sp[:],
                                     mybir.ActivationFunctionType.Exp,
                                     bias=bias_t[:])
                return pt

            def group_tail(g):
                # reciprocal of one 8-head sums group, bounced through DRAM
                # so broadcast-read DMAs can replicate each head's recip row
                # to its 64 partitions (gpsimd partition_broadcast is
                # unreliable on hw; DMA broadcast reads are exact).  Group 0
                # launches at head 7 so its 8 broadcast DMAs drain during
                # heads 8-15; group 1's drain during the next Qproj.  The
                # DMAs alternate between the gpsimd and scalar trigger
                # queues so neither queue sees a >4-deep burst.
                base = rt * 16 + g * 8
                rcf = rcfpool.tile([8, RT], dt.float32, tag="rcf")
                nc.scalar.activation(rcf[0:8, :], rt_su[0][0:8, :],
                                     mybir.ActivationFunctionType.Ln)
                rcb = rcbpool.tile([8, RT], dt.bfloat16, tag="rcb")
                nc.scalar.activation(rcb[0:8, :], rcf[0:8, :],
                                     mybir.ActivationFunctionType.Exp,
                                     scale=-1.0)
                nc.scalar.dma_start(out=rcb_d[base:base + 8, :],
                                    in_=rcb[0:8, :])
                for cc in range(4):
                    c = g * 4 + cc
                    rb = rbpool.tile([128, RT], dt.bfloat16, tag="rb")
                    eng = nc.gpsimd if cc % 2 == 0 else nc.scalar
                    eng.dma_start(
                        out=rb[0:64, :],
                        in_=rcb_d[base + 2 * cc:base + 2 * cc + 1, :]
                        .to_broadcast((64, RT)))
                    eng.dma_start(
                        out=rb[64:128, :],
                        in_=rcb_d[base + 2 * cc + 1:base + 2 * cc + 2, :]
                        .to_broadcast((64, RT)))
                    if rt < NRT - 1:
                        pending_muls.append((at_t, au_list[c], rb, c))
                    else:
                        nc.vector.tensor_mul(
                            at_t[:, c * RT:(c + 1) * RT], au_list[c][:], rb[:],
                        )

            def head_back(h, pt):
                po = (h % 2) * HD
                # head h's softmax sum -> row h%8 of its group's [128, RT]
                # tile (rows 8..127 accumulate zeros from the padded selector)
                if h % 8 == 0:
                    rt_su[0] = supsum.tile([128, RT], dt.float32, tag="su",
                                           name="su")
                nc.tensor.matmul(
                    rt_su[0][:],
                    lhsT=sel_t[:, h * 128:(h + 1) * 128],
                    rhs=pt[:],
                    start=(h % 8 == 0), stop=(h % 8 == 7),
                )
                if po == 0:
                    ap_ = apsum.tile([128, RT], dt.float32, tag="ap")
                    pair_ap[0] = ap_
                nc.tensor.matmul(
                    pair_ap[0][:],
                    lhsT=v64_t[:, h * 128:(h + 1) * 128],
                    rhs=pt[:],
                    start=(po == 0), stop=(po != 0),
                )
                if po != 0:
                    # pair complete: stash unnormalized A^T pair in SBUF fp32
                    au = aupool.tile([128, RT], dt.float32, tag="au")
                    nc.vector.tensor_copy(au[:], pair_ap[0][:])
                    au_list.append(au)
                if h % 8 == 7:
                    group_tail(h // 8)

            # software-pipelined head loop: exp(h) overlaps qk(h+1) and the
            # previous head's sums/PV plus a Wproj batch
            prev_pt = None
            for h in range(H):
                pt = head_front(h)
                if prev_pt is not None:
                    head_back(h - 1, prev_pt)
                prev_pt = pt
                if h >= 2:
                    drain(1)
            head_back(H - 1, prev_pt)
            drain(99)

            prev = (rt, at_t)

        # drain: Wproj of the final rowtile
        for _ in wproj_gen(prev[0], prev[1], split_dma=True):
            pass

    nc.compile()
    return nc


def _get_program():
    if "p" not in _PROG_CACHE:
        _PROG_CACHE["p"] = _build_program()
    return _PROG_CACHE["p"]


def _prep_inputs(x, te, mask, Wq, Wk, Wv, Wo, Wst):
    """Host-side fp32 weight prep + per-core shard maps."""
    K = (te @ Wk).reshape(B, TT, G, HD) * SCALE
    V = (te @ Wv).reshape(B, TT, G, HD)
    Weff = ((Wst[:D] + Wst[D:]) @ Wo).astype(np.float32)
    weff_b = Weff.astype(BF16)

    # fp8 half of Wq (chunks 0-3): wq8[p, cc, j, i, m] = Wq[(2j+i)*128+p, cc*128+m]
    wq8_b = np.ascontiguousarray(
        Wq[:512].reshape(2, 2, 128, NK, 128).transpose(2, 3, 0, 1, 4)
    ).astype(FP8)
    # bf16 half (chunks 4-7): wqb[p, cc, k*128+m] = Wq[(4+k)*128+p, cc*128+m]
    wqb_b = np.ascontiguousarray(
        Wq[512:].reshape(NKB, 128, NK, 128).transpose(1, 2, 0, 3).reshape(
            128, NK, NKB * 128)
    ).astype(BF16)

    # per-head one-hot sum selectors [128, H*128]: block h col h%8 = 1 on
    # rows 0..76 (two 8-head groups, each summing into rows 0..7 of its tile)
    sel_np = np.zeros((128, H * 128), np.float32)
    for h in range(H):
        sel_np[0:TT, h * 128 + h % 8] = 1.0
    sel_b = sel_np.astype(BF16)

    kt_b, v_b, bias_b = [], [], []
    for b in range(B):
        # qk blocks [128, H*128]: block h rows (h%2)*64..+64, cols 0..76 = K_g^T
        ktq = np.zeros((128, H * 128), np.float32)
        # PV blocks [128, H*128]: block h rows 0..76, cols (h%2)*64..+64 = V_g
        vp = np.zeros((128, H * 128), np.float32)
        for h in range(H):
            g = h // HPG
            po = (h % 2) * HD
            ktq[po:po + HD, h * 128:h * 128 + TT] = K[b, :, g, :].T
            vp[0:TT, h * 128 + po:h * 128 + po + HD] = V[b, :, g, :]
        kt_b.append(ktq.astype(BF16))
        v_b.append(vp.astype(BF16))
        bv = np.zeros((128, 1), np.float32)
        bv[0:TT, 0] = np.where(mask[b], 0.0, -30.0)
        bias_b.append(bv)

    in_maps = []
    for c in range(NCORES):
        b = c // (NCORES // B)
        fr = (c % (NCORES // B)) * FPC
        xc = x[b, fr:fr + FPC].reshape(ROWS, D)
        # xt8[p, j, i, r] = X[r, (2j+i)*128+p] for chunks 0-3
        xt8_c = np.ascontiguousarray(
            xc[:, :512].T.reshape(2, 2, 128, ROWS)
            .transpose(2, 0, 1, 3)).astype(FP8)
        # xtb[p, k, r] = X[r, (4+k)*128+p]
        xtb_c = np.ascontiguousarray(
            xc[:, 512:].T.reshape(NKB, 128, ROWS).transpose(1, 0, 2)
        ).astype(BF16)
        m = {
            "xt8": xt8_c,
            "xtb": xtb_c,
            "wq8": wq8_b,
            "wqb": wqb_b,
            "weff": weff_b,
            "kt": kt_b[b],
            "v64": v_b[b],
            "sel": sel_b,
            "biasr": bias_b[b],
        }
        in_maps.append(m)
    return in_maps


def kernel(x, text_embeddings, padding_mask, use_mqa=0, use_qk_norm=0,
           Wq=None, Wk=None, Wv=None, Wo=None, Wst=None):
    global LAST_RESULTS
    x = np.asarray(x, np.float32)
    te = np.asarray(text_embeddings, np.float32)
    mask = np.asarray(padding_mask).astype(bool)
    Wq = np.asarray(Wq, np.float32)
    Wk = np.asarray(Wk, np.float32)
    Wv = np.asarray(Wv, np.float32)
    Wo = np.asarray(Wo, np.float32)
    Wst = np.asarray(Wst, np.float32)
    assert x.shape == (B, T, HW, D) and te.shape == (B, TT, D)

    in_maps = _prep_inputs(x, te, mask, Wq, Wk, Wv, Wo, Wst)
    nc = _get_program()

    res = run_bass_kernel_spmd(nc, in_maps, list(range(NCORES)),
                               trace=TRACE, **TRACE_KWARGS)
    LAST_RESULTS = res

    outp = np.empty((B, T, HW, D), np.float32)
    for c in range(NCORES):
        b = c // (NCORES // B)
        fr = (c % (NCORES // B)) * FPC
        outp[b, fr:fr + FPC] = res.results[c]["out"].reshape(FPC, HW, D)
    return outp
